# revision 8
# baseline (speedup 1.0000x reference)
"""DropStripes Trainium2 kernel.

out[b, t, f] = x[b, t, f] * keep[b, f], where keep[b, f] = 0 iff f falls in
any stripe [bgn[b,s], bgn[b,s]+distance[b,s]) for s in range(STRIPES).

Strategy: pure data-parallel over the batch dim (64 batches -> 8 cores x 8).
The (B, F) keep mask is expanded from the tiny (B, S) index arrays on the
host; each core then streams its 8 batches through SBUF: one 4 MB load per
batch (125 partitions x 16 rows x 512 f32, contiguous per partition), an
in-place DVE multiply against the per-batch mask row (partition-broadcast,
stride-0 repeat across the 16-row dim), one 4 MB store.
"""

import sys

if "/opt/trn_rl_repo" not in sys.path:
    sys.path.insert(0, "/opt/trn_rl_repo")

import numpy as np

B, T, F = 64, 2000, 512
N_CORES = 8
BPC = B // N_CORES  # batches per core
P = 125  # SBUF partitions used (125 * 16 = 2000 rows)
K = T // P  # rows of F per partition

_cached = {}


def _demote_deps(bass_ins, keep_names):
    """Keep only `keep_names` as semaphore-wait (sync) deps; demote the rest
    to nosync (scheduler-ordering-only) deps.

    The DVE TensorTensor ISA slot can't hold 3+ sync waits, and Tile's sem
    pass is not transitively minimal: the multiply would wait on its load,
    on the store that freed its SBUF slot (already implied by the load's own
    WAR wait), and on an earlier same-engine DVE op (implied by in-order
    execution). Demotion preserves scheduler ordering, so the implication
    chains stay valid.
    """
    from concourse.instruction_name_ordered_set import InstructionNameOrderedSet

    ins = bass_ins.ins
    cur = ins.sync_dependency_set_copy()
    keep = InstructionNameOrderedSet([n for n in cur if n in keep_names])
    demote = cur.difference(keep)
    ins.set_sync_dependencies(keep)
    ins.add_nosync_dependencies_from(demote)


def _build_program():
    import concourse.bass as bass
    import concourse.mybir as mybir
    from concourse.tile import TileContext

    F32 = mybir.dt.float32
    nc = bass.Bass()

    x = nc.dram_tensor("x", [BPC, T, F], F32, kind="ExternalInput")
    # Host pre-replicates each batch's keep-mask row across the 125 SBUF
    # partitions: mask[p, b*F + f] = keep[b, f].
    mask = nc.dram_tensor("mask", [P, BPC * F], F32, kind="ExternalInput")
    out = nc.dram_tensor("out", [BPC, T, F], F32, kind="ExternalOutput")

    NBUF = 4
    loads, tts, stores = [], [], []
    with TileContext(nc) as tc:
        with (
            tc.tile_pool(name="xp", bufs=NBUF) as xp,
            tc.tile_pool(name="mp", bufs=1) as mp,
        ):
            m = mp.tile([P, BPC * F], F32)
            mask_ld = nc.sync.dma_start(out=m[:], in_=mask[:])
            for b in range(BPC):
                t = xp.tile([P, K * F], F32)
                ld = nc.sync.dma_start(
                    out=t[:], in_=x[b].rearrange("(p k) f -> p (k f)", p=P)
                )
                # load(b) only needs the WAR on the store that last read its
                # slot; everything else is ordering-only.
                ld_keep = {stores[b - NBUF].ins.name} if b >= NBUF else set()
                _demote_deps(ld, ld_keep)

                t3 = t[:].rearrange("p (k f) -> p k f", f=F)
                mb = m[:, b * F : (b + 1) * F]
                tt = nc.vector.tensor_tensor(
                    out=t3,
                    in0=t3,
                    in1=mb[:, None, :].to_broadcast((P, K, F)),
                    op=mybir.AluOpType.mult,
                )
                _demote_deps(tt, {ld.ins.name})

                # Store from ACT's HWDGE ring so its wait on the multiply
                # doesn't stall the next batch's load on SP's ring.
                st = nc.scalar.dma_start(
                    out=out[b].rearrange("(p k) f -> p (k f)", p=P), in_=t[:]
                )
                _demote_deps(st, {tt.ins.name})
                loads.append(ld)
                tts.append(tt)
                stores.append(st)

    # Post-scheduling wait minimization. The DVE TensorTensor ISA slot holds
    # only ONE sync wait, and Tile's slot allocator re-attaches recycling
    # waits after scheduling, so each instruction is pruned to its provably
    # minimal wait:
    #   TT(b)    <- load(b)'s DMA-lane sem only. The load itself carries the
    #               slot-WAR waits, and the one-time mask DMA precedes the
    #               loads on the same qSPDynamicHW ring (per-engine FIFO =>
    #               load(b) complete implies mask complete).
    #   store(b) <- the DVE sem only (TT(b) complete implies everything).
    #   load(b)  <- store(b-NBUF)'s DMA-lane sem only (slot WAR; earlier
    #               same-ring loads are FIFO-implied).
    def _lane(dma_bass_ins):
        upds = dma_bass_ins.ins.sync_info.on_update
        assert len(upds) == 1, upds
        return upds[0].ant_name

    def _keep_waits(bass_ins, pred, expect=True):
        ins = bass_ins.ins
        si = ins.sync_info
        if si is None:
            assert not expect, f"{ins.name}: no sync_info"
            return
        kept = [w for w in si.on_wait if pred(w)]
        if expect:
            assert kept, f"{ins.name}: expected wait missing from {si.on_wait}"
        ins.sync_info = mybir.SyncInfo(on_wait=kept, on_update=si.on_update)

    for b in range(BPC):
        ld_lane = _lane(loads[b])
        _keep_waits(tts[b], lambda w, s=ld_lane: w.ant_name == s)
        _keep_waits(
            stores[b], lambda w: (w.ant_name or "").startswith("DVE")
        )
        if b >= NBUF:
            st_lane = _lane(stores[b - NBUF])
            _keep_waits(loads[b], lambda w, s=st_lane: w.ant_name == s)
        else:
            _keep_waits(loads[b], lambda w: False, expect=False)

    # The kernel-tail Drain waits on every DMA lane + the DVE sem (9 waits,
    # over the CTRL ISA wait capacity). The last store's lane alone implies
    # all of it: store(7) <- TT(7) <- load(7), earlier stores are FIFO-
    # ordered on the same HWDGE ring, and earlier loads feed earlier stores.
    last_lane = _lane(stores[-1])
    for bb in nc.main_func.blocks:
        for ins in bb.instructions:
            if type(ins).__name__ != "InstDrain":
                continue
            si = ins.sync_info
            if not si or len(si.on_wait) <= 1:
                continue
            kept = [w for w in si.on_wait if w.ant_name == last_lane]
            assert kept, f"{ins.name}: no wait on {last_lane} in {si.on_wait}"
            ins.sync_info = mybir.SyncInfo(on_wait=kept, on_update=si.on_update)
    return nc


def _expand_mask(bgn: np.ndarray, distance: np.ndarray) -> np.ndarray:
    pos = np.arange(F)
    bgn = np.asarray(bgn).astype(np.int64)
    dist = np.asarray(distance).astype(np.int64)
    in_stripe = (pos[None, None, :] >= bgn[:, :, None]) & (
        pos[None, None, :] < (bgn + dist)[:, :, None]
    )
    keep = ~np.any(in_stripe, axis=1)  # (B, F)
    return keep.astype(np.float32)


def kernel(x, bgn, distance, _trace=False, _trace_kwargs=None):
    from concourse.bass_utils import run_bass_kernel_spmd

    x = np.ascontiguousarray(np.asarray(x, dtype=np.float32))
    keep = _expand_mask(bgn, distance)

    if "nc" not in _cached:
        _cached["nc"] = _build_program()
    nc = _cached["nc"]

    in_maps = []
    for i in range(N_CORES):
        sl = slice(i * BPC, (i + 1) * BPC)
        # (BPC, F) -> (P, BPC*F): each partition row holds all BPC mask rows.
        mask_rep = np.ascontiguousarray(
            np.broadcast_to(keep[sl].reshape(1, BPC * F), (P, BPC * F))
        )
        in_maps.append({"x": x[sl], "mask": mask_rep})

    res = run_bass_kernel_spmd(
        nc, in_maps, list(range(N_CORES)), trace=_trace, **(_trace_kwargs or {})
    )
    _cached["last_results"] = res
    return np.concatenate([r["out"] for r in res.results], axis=0)


# revision 9
# speedup vs baseline: 1.6392x; 1.6392x over previous
"""DropStripes Trainium2 kernel.

out[b, t, f] = x[b, t, f] * keep[b, f], where keep[b, f] = 0 iff f falls in
any stripe [bgn[b,s], bgn[b,s]+distance[b,s]) for s in range(STRIPES).

Strategy: pure data-parallel over the batch dim (64 batches -> 8 cores x 8).
The (B, F) keep mask is expanded from the tiny (B, S) index arrays on the
host; each core then streams its 8 batches through SBUF: one 4 MB load per
batch (125 partitions x 16 rows x 512 f32, contiguous per partition), an
in-place DVE multiply against the per-batch mask row (partition-broadcast,
stride-0 repeat across the 16-row dim), one 4 MB store.
"""

import sys

if "/opt/trn_rl_repo" not in sys.path:
    sys.path.insert(0, "/opt/trn_rl_repo")

import numpy as np

B, T, F = 64, 2000, 512
N_CORES = 8
BPC = B // N_CORES  # batches per core
P = 125  # SBUF partitions used (125 * 16 = 2000 rows)
K = T // P  # rows of F per partition

_cached = {}


def _demote_deps(bass_ins, keep_names):
    """Keep only `keep_names` as semaphore-wait (sync) deps; demote the rest
    to nosync (scheduler-ordering-only) deps.

    The DVE TensorTensor ISA slot can't hold 3+ sync waits, and Tile's sem
    pass is not transitively minimal: the multiply would wait on its load,
    on the store that freed its SBUF slot (already implied by the load's own
    WAR wait), and on an earlier same-engine DVE op (implied by in-order
    execution). Demotion preserves scheduler ordering, so the implication
    chains stay valid.
    """
    from concourse.instruction_name_ordered_set import InstructionNameOrderedSet

    ins = bass_ins.ins
    cur = ins.sync_dependency_set_copy()
    keep = InstructionNameOrderedSet([n for n in cur if n in keep_names])
    demote = cur.difference(keep)
    ins.set_sync_dependencies(keep)
    ins.add_nosync_dependencies_from(demote)


def _build_program():
    import concourse.bass as bass
    import concourse.mybir as mybir
    from concourse.tile import TileContext

    F32 = mybir.dt.float32
    nc = bass.Bass()

    x = nc.dram_tensor("x", [BPC, T, F], F32, kind="ExternalInput")
    # Host pre-replicates each batch's keep-mask row across the 125 SBUF
    # partitions: mask[p, b*F + f] = keep[b, f].
    mask = nc.dram_tensor("mask", [P, BPC * F], F32, kind="ExternalInput")
    out = nc.dram_tensor("out", [BPC, T, F], F32, kind="ExternalOutput")

    # All bulk DMAs go through SWDGE (gpsimd): this runtime fans one HWDGE
    # DMA over only 5 SDMA engines (~135 GB/s for loads+stores combined),
    # while SWDGE sprays descriptors across all 16. Since every DMA then
    # issues from the single POOL engine, the loop is software-pipelined by
    # hand: load(b+PF) is issued BEFORE store(b), so the store's wait on the
    # multiply never stalls upcoming loads.
    NBUF = 4
    PF = 2  # prefetch depth
    loads, tts, stores = [], [], []

    def _mk_load(b, tiles, xp):
        t = xp.tile([P, K * F], F32)
        ld = nc.gpsimd.dma_start(
            out=t[:], in_=x[b].rearrange("(p k) f -> p (k f)", p=P)
        )
        ld_keep = {stores[b - NBUF].ins.name} if b >= NBUF else set()
        _demote_deps(ld, ld_keep)
        loads.append(ld)
        tiles[b] = t

    with TileContext(nc) as tc:
        with (
            tc.tile_pool(name="xp", bufs=NBUF) as xp,
            tc.tile_pool(name="mp", bufs=1) as mp,
        ):
            m = mp.tile([P, BPC * F], F32)
            mask_ld = nc.gpsimd.dma_start(out=m[:], in_=mask[:])
            tiles = {}
            for b in range(min(PF, BPC)):
                _mk_load(b, tiles, xp)
            for b in range(BPC):
                if b + PF < BPC:
                    _mk_load(b + PF, tiles, xp)
                t = tiles.pop(b)
                t3 = t[:].rearrange("p (k f) -> p k f", f=F)
                mb = m[:, b * F : (b + 1) * F]
                tt = nc.vector.tensor_tensor(
                    out=t3,
                    in0=t3,
                    in1=mb[:, None, :].to_broadcast((P, K, F)),
                    op=mybir.AluOpType.mult,
                )
                _demote_deps(tt, {loads[b].ins.name})

                st = nc.gpsimd.dma_start(
                    out=out[b].rearrange("(p k) f -> p (k f)", p=P), in_=t[:]
                )
                _demote_deps(st, {tt.ins.name})
                tts.append(tt)
                stores.append(st)

    # Post-scheduling wait minimization. The DVE TensorTensor ISA slot holds
    # only ONE sync wait, and Tile's slot allocator re-attaches recycling
    # waits after scheduling, so each instruction is pruned to its provably
    # minimal wait:
    #   TT(b)    <- load(b)'s DMA-lane sem only. The load itself carries the
    #               slot-WAR waits, and the one-time mask DMA precedes the
    #               loads on the same qSPDynamicHW ring (per-engine FIFO =>
    #               load(b) complete implies mask complete).
    #   store(b) <- the DVE sem only (TT(b) complete implies everything).
    #   load(b)  <- store(b-NBUF)'s DMA-lane sem only (slot WAR; earlier
    #               same-ring loads are FIFO-implied).
    def _lane(dma_bass_ins):
        upds = dma_bass_ins.ins.sync_info.on_update
        assert len(upds) == 1, upds
        return upds[0].ant_name

    def _keep_waits(bass_ins, pred, expect=True):
        ins = bass_ins.ins
        si = ins.sync_info
        if si is None:
            assert not expect, f"{ins.name}: no sync_info"
            return
        kept = [w for w in si.on_wait if pred(w)]
        if expect:
            assert kept, f"{ins.name}: expected wait missing from {si.on_wait}"
        ins.sync_info = mybir.SyncInfo(on_wait=kept, on_update=si.on_update)

    for b in range(BPC):
        ld_lane = _lane(loads[b])
        _keep_waits(tts[b], lambda w, s=ld_lane: w.ant_name == s)
        _keep_waits(
            stores[b], lambda w: (w.ant_name or "").startswith("DVE")
        )
        if b >= NBUF:
            st_lane = _lane(stores[b - NBUF])
            _keep_waits(loads[b], lambda w, s=st_lane: w.ant_name == s)
        else:
            _keep_waits(loads[b], lambda w: False, expect=False)

    # The kernel-tail Drain waits on every DMA lane + the DVE sem (9 waits,
    # over the CTRL ISA wait capacity). The last store's lane alone implies
    # all of it: store(7) <- TT(7) <- load(7), earlier stores are FIFO-
    # ordered on the same HWDGE ring, and earlier loads feed earlier stores.
    last_lane = _lane(stores[-1])
    for bb in nc.main_func.blocks:
        for ins in bb.instructions:
            if type(ins).__name__ != "InstDrain":
                continue
            si = ins.sync_info
            if not si or len(si.on_wait) <= 1:
                continue
            kept = [w for w in si.on_wait if w.ant_name == last_lane]
            assert kept, f"{ins.name}: no wait on {last_lane} in {si.on_wait}"
            ins.sync_info = mybir.SyncInfo(on_wait=kept, on_update=si.on_update)
    return nc


def _expand_mask(bgn: np.ndarray, distance: np.ndarray) -> np.ndarray:
    pos = np.arange(F)
    bgn = np.asarray(bgn).astype(np.int64)
    dist = np.asarray(distance).astype(np.int64)
    in_stripe = (pos[None, None, :] >= bgn[:, :, None]) & (
        pos[None, None, :] < (bgn + dist)[:, :, None]
    )
    keep = ~np.any(in_stripe, axis=1)  # (B, F)
    return keep.astype(np.float32)


def kernel(x, bgn, distance, _trace=False, _trace_kwargs=None):
    from concourse.bass_utils import run_bass_kernel_spmd

    x = np.ascontiguousarray(np.asarray(x, dtype=np.float32))
    keep = _expand_mask(bgn, distance)

    if "nc" not in _cached:
        _cached["nc"] = _build_program()
    nc = _cached["nc"]

    in_maps = []
    for i in range(N_CORES):
        sl = slice(i * BPC, (i + 1) * BPC)
        # (BPC, F) -> (P, BPC*F): each partition row holds all BPC mask rows.
        mask_rep = np.ascontiguousarray(
            np.broadcast_to(keep[sl].reshape(1, BPC * F), (P, BPC * F))
        )
        in_maps.append({"x": x[sl], "mask": mask_rep})

    res = run_bass_kernel_spmd(
        nc, in_maps, list(range(N_CORES)), trace=_trace, **(_trace_kwargs or {})
    )
    _cached["last_results"] = res
    return np.concatenate([r["out"] for r in res.results], axis=0)
